# revision 9
# baseline (speedup 1.0000x reference)
"""AttnDecoderRNN single-step decoder on 8 TRN2 NeuronCores.

Sharding: data-parallel over batch B=64 -> 8 rows/core for the
LSTM + additive attention (the dominant compute, the L*B*H*H attention
projection, is evenly split); vocab-tensor-parallel for the output
projection (W_out split into 8 slices of 4000 rows). Cross-core
traffic: AllGather of co (16KB/core) + AllReduce of the softmax
denominator partials (256B).

Device layouts (host pre-stages):
  encT  [H=512, L*8=8192]  encoder slice transposed, lb = l*8 + b
  all matmul operands typed float32r (4x PE throughput, ~1.5e-4 rel err)
"""
import sys

if "/opt/trn_rl_repo" not in sys.path:
    sys.path.insert(0, "/opt/trn_rl_repo")

import contextlib

import numpy as np

import concourse.bass as bass
import concourse.tile as tile
from concourse import bacc, mybir
from concourse.bass_utils import run_bass_kernel_spmd

F32 = mybir.dt.float32
F32R = mybir.dt.float32r
I32 = mybir.dt.int32
AF = mybir.ActivationFunctionType
ALU = mybir.AluOpType

N_CORES = 8
B, H, E, V, L = 64, 512, 256, 32000, 1024
BL = B // N_CORES          # 8 batch rows per core
VS = V // N_CORES          # 4000 vocab rows per core
HC = H // 128              # 4 h chunks
EC = E // 128              # 2 embedding chunks


def _build(nL=L):
    """Build the SPMD kernel. nL parameterizes seq len (small for sim)."""
    LBL = nL * BL            # local (l, b) row count
    NG = max(1, LBL // 512)  # lb groups
    GW = LBL // NG           # group width (512 normally)
    NV = VS // 500           # 8 vocab N-chunks of 500

    nc = bacc.Bacc("TRN2", target_bir_lowering=False, debug=False,
                   num_devices=N_CORES)
    dram = lambda n, s, d: nc.dram_tensor(n, s, d, kind="ExternalInput").ap()
    outd = lambda n, s: nc.dram_tensor(n, s, F32, kind="ExternalOutput").ap()

    # inputs
    encT = dram("encT", [H, LBL], F32R)
    tok_idx = dram("tok_idx", [BL, 1], I32)
    emb = dram("emb", [V, E], F32)
    locT = dram("locT", [H, BL], F32R)
    hprevT = dram("hprevT", [H, BL], F32R)
    cprev = dram("cprev", [BL, H], F32)
    WihT = dram("WihT", [E + H, 4 * H], F32R)
    bihh = dram("bihh", [1, 4 * H], F32R)
    WhhT = dram("WhhT", [H, 4 * H], F32R)
    WhT = dram("WhT", [H, H], F32R)
    battn = dram("battn", [1, H], F32R)
    WencT = dram("WencT", [H, H], F32R)
    beta = dram("beta", [H, 1], F32R)
    WocT = dram("WocT", [2 * H, H], F32R)
    boc = dram("boc", [1, H], F32R)
    WoutT = dram("WoutT", [H, VS], F32R)
    bout = dram("bout", [1, VS], F32R)
    maskd = dram("maskd", [BL, GW], F32R)      # mask[b, f] = (f % 8 == b)
    ones128 = dram("ones128", [1, 128], F32R)
    ident = dram("ident", [128, 128], F32)

    # outputs
    out_slice = outd("out_slice", [B, VS])
    co_out = outd("co_out", [BL, H])
    h_out = outd("h_out", [BL, H])
    c_out = outd("c_out", [BL, H])

    with tile.TileContext(nc) as tc, contextlib.ExitStack() as ctx:
        sb = ctx.enter_context(tc.tile_pool(name="sb", bufs=1))
        sbs = ctx.enter_context(tc.tile_pool(name="sbs", bufs=2))
        enc_pool = ctx.enter_context(tc.tile_pool(name="encp", bufs=2))
        tanh_pool = ctx.enter_context(tc.tile_pool(name="tanhp", bufs=2))
        w_pool = ctx.enter_context(tc.tile_pool(name="wp", bufs=2))
        wout_pool = ctx.enter_context(tc.tile_pool(name="woutp", bufs=1))
        ps = ctx.enter_context(tc.tile_pool(name="ps", bufs=1, space="PSUM"))
        dr = ctx.enter_context(tc.tile_pool(name="dr", bufs=1, space="DRAM"))

        _pid = [0]

        def psum(shape, tag="mmO", bufs=5):
            _pid[0] += 1
            return ps.tile(shape, F32, space="PSUM", tag=tag, bufs=bufs,
                           name=f"ps_{tag}_{_pid[0]}")

        # ---- constants / small resident tiles ----
        identt = sb.tile([128, 128], F32)
        nc.sync.dma_start(identt[:], ident[:])
        ones_t = sb.tile([1, 128], F32R)
        nc.sync.dma_start(ones_t[:], ones128[:])
        mask_t = sb.tile([BL, GW], F32R)
        nc.sync.dma_start(mask_t[:], maskd[:])
        beta_c = []
        for k in range(HC):
            bt = sb.tile([128, 1], F32R, tag=f"beta{k}", name=f"beta{k}")
            nc.sync.dma_start(bt[:], beta[k * 128:(k + 1) * 128, :])
            beta_c.append(bt)
        WencT_c = []
        for k in range(HC):
            row = []
            for m in range(HC):
                wt = sb.tile([128, 128], F32R, tag=f"wenc{k}{m}",
                             name=f"wenc{k}{m}")
                nc.sync.dma_start(
                    wt[:], WencT[k * 128:(k + 1) * 128,
                                 m * 128:(m + 1) * 128])
                row.append(wt)
            WencT_c.append(row)

        # ---- phase 0: embedding gather + LSTM + h-term ----
        idx_t = sbs.tile([BL, 1], I32, bufs=1)
        nc.sync.dma_start(idx_t[:], tok_idx[:])
        tok_t = sbs.tile([BL, E], F32, bufs=1)
        nc.gpsimd.indirect_dma_start(
            out=tok_t[:], out_offset=None, in_=emb[:],
            in_offset=bass.IndirectOffsetOnAxis(ap=idx_t[:, :1], axis=0))

        # xT chunks (f32r): tok (2 via PE transpose), loc (4 from DRAM)
        xT = []
        for k in range(EC):
            tp = psum([128, BL])
            nc.tensor.transpose(tp[:], tok_t[:, k * 128:(k + 1) * 128],
                                identt[:BL, :BL])
            xt = sbs.tile([128, BL], F32R, tag="xT", bufs=EC + HC,
                          name=f"xT{k}")
            nc.scalar.copy(xt[:], tp[:])
            xT.append(xt)
        for k in range(HC):
            xt = sbs.tile([128, BL], F32R, tag="xT", bufs=EC + HC,
                          name=f"xT{EC + k}")
            nc.sync.dma_start(xt[:], locT[k * 128:(k + 1) * 128, :])
            xT.append(xt)
        hpv = []
        for k in range(HC):
            ht = sbs.tile([128, BL], F32R, tag="hprevT", bufs=HC,
                          name=f"hprevT{k}")
            nc.sync.dma_start(ht[:], hprevT[k * 128:(k + 1) * 128, :])
            hpv.append(ht)

        # gates[b, 4H] in 4 psum tiles [8, 512] (i, f, g, o)
        gact = []
        for n in range(4):
            gp = psum([BL, H])
            nsl = slice(n * H, (n + 1) * H)
            for k in range(EC + HC):   # W_ih part
                wt = w_pool.tile([128, H], F32R, tag="wih", name=f"wih{n}_{k}")
                nc.sync.dma_start(wt[:], WihT[k * 128:(k + 1) * 128, nsl])
                nc.tensor.matmul(gp[:], xT[k][:], wt[:],
                                 start=(k == 0), stop=False)
            for k in range(HC):        # W_hh part
                wt = w_pool.tile([128, H], F32R, tag="whh", name=f"whh{n}_{k}")
                nc.sync.dma_start(wt[:], WhhT[k * 128:(k + 1) * 128, nsl])
                nc.tensor.matmul(gp[:], hpv[k][:], wt[:],
                                 start=False, stop=False)
            bt = w_pool.tile([1, H], F32R, tag="bias", name=f"bias{n}")
            nc.sync.dma_start(bt[:], bihh[:, nsl])
            nc.tensor.matmul(gp[:], ones_t[:, :BL], bt[:],
                             start=False, stop=True)
            ga = sbs.tile([BL, H], F32, tag="gate", bufs=4, name=f"gate{n}")
            nc.scalar.activation(ga[:], gp[:],
                                 AF.Tanh if n == 2 else AF.Sigmoid)
            gact.append(ga)

        cprev_t = sbs.tile([BL, H], F32, bufs=1)
        nc.sync.dma_start(cprev_t[:], cprev[:])
        t1 = sbs.tile([BL, H], F32, bufs=1)
        nc.vector.tensor_mul(t1[:], gact[1][:], cprev_t[:])   # f * c_prev
        t2 = sbs.tile([BL, H], F32, bufs=1)
        nc.vector.tensor_mul(t2[:], gact[0][:], gact[2][:])   # i * g
        c_new = sbs.tile([BL, H], F32, bufs=1)
        nc.vector.tensor_add(c_new[:], t1[:], t2[:])
        nc.sync.dma_start(c_out[:], c_new[:])
        ctanh = sbs.tile([BL, H], F32, bufs=1)
        nc.scalar.activation(ctanh[:], c_new[:], AF.Tanh)
        h_new = sbs.tile([BL, H], F32, bufs=1)
        nc.vector.tensor_mul(h_new[:], gact[3][:], ctanh[:])  # o * tanh(c)
        nc.sync.dma_start(h_out[:], h_new[:])

        # hT chunks (f32r), used by hp and co matmuls
        hT = []
        for k in range(HC):
            tp = psum([128, BL])
            nc.tensor.transpose(tp[:], h_new[:, k * 128:(k + 1) * 128],
                                identt[:BL, :BL])
            ht = sbs.tile([128, BL], F32R, tag="hT", bufs=HC, name=f"hT{k}")
            nc.scalar.copy(ht[:], tp[:])
            hT.append(ht)

        # hp[b, h_out] = h @ W_h.T + b_attn
        WhT_c = []
        for k in range(HC):
            wt = sbs.tile([128, H], F32R, tag="whT", bufs=2, name=f"whT{k}")
            nc.sync.dma_start(wt[:], WhT[k * 128:(k + 1) * 128, :])
            WhT_c.append(wt)
        battn_t = sbs.tile([1, H], F32R, bufs=1)
        nc.sync.dma_start(battn_t[:], battn[:])
        hpp = psum([BL, H])
        for k in range(HC):
            nc.tensor.matmul(hpp[:], hT[k][:], WhT_c[k][:],
                             start=(k == 0), stop=False)
        nc.tensor.matmul(hpp[:], ones_t[:, :BL], battn_t[:],
                         start=False, stop=True)
        hp_t = sbs.tile([BL, H], F32R, bufs=1)
        nc.scalar.copy(hp_t[:], hpp[:])

        # context accumulators + softmax running sum
        ctx_acc = []
        for m in range(HC):
            ca = sb.tile([128, BL], F32, tag=f"ctx{m}", name=f"ctx{m}")
            nc.gpsimd.memset(ca[:], 0.0)
            ctx_acc.append(ca)
        sum_acc = sb.tile([1, BL], F32)
        nc.gpsimd.memset(sum_acc[:], 0.0)

        # WoutT tiles: prefetched during the attention loop
        wout_tiles = {}

        def prefetch_wout(j):
            if j >= HC * NV:
                return
            k, n = divmod(j, NV)
            wt = wout_pool.tile([128, 500], F32R, tag=f"wo{j}", name=f"wo{j}")
            nc.sync.dma_start(
                wt[:], WoutT[k * 128:(k + 1) * 128, n * 500:(n + 1) * 500])
            wout_tiles[(k, n)] = wt

        # ---- phase 1: attention main loop over lb groups ----
        for g in range(NG):
            gsl = slice(g * GW, (g + 1) * GW)
            enc_g = []
            for k in range(HC):
                et = enc_pool.tile([128, GW], F32R, tag=f"enc{k}",
                                   name=f"enc{k}_{g}")
                nc.sync.dma_start(et[:], encT[k * 128:(k + 1) * 128, gsl])
                enc_g.append(et)
            tanh_g = []
            for m in range(HC):
                ep = psum([128, GW])
                for k in range(HC):
                    nc.tensor.matmul(ep[:], WencT_c[k][m][:], enc_g[k][:],
                                     start=(k == 0), stop=False)
                nc.tensor.matmul(ep[:], hp_t[:, m * 128:(m + 1) * 128],
                                 mask_t[:], start=False, stop=True)
                tt = tanh_pool.tile([128, GW], F32R, tag=f"tanh{m}",
                                    name=f"tanh{m}_{g}")
                nc.scalar.activation(tt[:], ep[:], AF.Tanh)
                tanh_g.append(tt)
            bp = psum([1, GW], tag="betaO", bufs=3)
            for m in range(HC):
                nc.tensor.matmul(bp[:], beta_c[m][:], tanh_g[m][:],
                                 start=(m == 0), stop=(m == HC - 1))
            exp_row = sbs.tile([1, GW], F32R, tag="exprow", name=f"exprow{g}")
            nc.scalar.activation(exp_row[:], bp[:], AF.Exp)
            # running softmax denominator (per b)
            spart = sbs.tile([1, BL], F32, tag="spart", name=f"spart{g}")
            nc.vector.tensor_reduce(
                spart[:],
                exp_row[:].bitcast(F32).rearrange("p (l b) -> p b l", b=BL),
                axis=mybir.AxisListType.X, op=ALU.add)
            nc.vector.tensor_add(sum_acc[:], sum_acc[:], spart[:])
            # replicate exp across partitions (PE K=1 ones-matmul)
            arep = psum([128, GW], tag="betaO", bufs=3)
            nc.tensor.matmul(arep[:], ones_t[:], exp_row[:],
                             start=True, stop=True)
            for m in range(HC):
                prod = sbs.tile([128, GW], F32, tag="prod",
                                name=f"prod{m}_{g}")
                nc.vector.tensor_mul(prod[:], enc_g[m][:].bitcast(F32),
                                     arep[:])
                cpart = sbs.tile([128, BL], F32, tag="cpart",
                                 name=f"cpart{m}_{g}")
                nc.vector.tensor_reduce(
                    cpart[:], prod[:].rearrange("p (l b) -> p b l", b=BL),
                    axis=mybir.AxisListType.X, op=ALU.add)
                nc.vector.tensor_add(ctx_acc[m][:], ctx_acc[m][:], cpart[:])
            prefetch_wout(2 * g)
            prefetch_wout(2 * g + 1)

        for j in range(2 * NG, HC * NV):
            prefetch_wout(j)

        # ---- phase 2: normalize context, co, exchange, logits ----
        inv1 = sbs.tile([1, BL], F32, bufs=1)
        nc.vector.reciprocal(inv1[:], sum_acc[:])
        inv1r = sbs.tile([1, BL], F32R, bufs=1)
        nc.scalar.copy(inv1r[:], inv1[:])
        irp = psum([128, BL])
        nc.tensor.matmul(irp[:], ones_t[:], inv1r[:], start=True, stop=True)
        inv_rep = sbs.tile([128, BL], F32, bufs=1)
        nc.scalar.copy(inv_rep[:], irp[:])
        ctxT = []
        for m in range(HC):
            cn = sbs.tile([128, BL], F32, tag="ctxn", name=f"ctxn{m}")
            nc.vector.tensor_mul(cn[:], ctx_acc[m][:], inv_rep[:])
            cnr = sbs.tile([128, BL], F32R, tag="ctxnr", bufs=HC,
                           name=f"ctxnr{m}")
            nc.scalar.copy(cnr[:], cn[:])
            ctxT.append(cnr)

        # co = tanh([h; ctx] @ W_oc.T + b_oc)
        WocT_c = []
        for k in range(2 * HC):
            wt = sbs.tile([128, H], F32R, tag="wocT", bufs=3,
                          name=f"wocT{k}")
            nc.sync.dma_start(wt[:], WocT[k * 128:(k + 1) * 128, :])
            WocT_c.append(wt)
        boc_t = sbs.tile([1, H], F32R, bufs=1)
        nc.sync.dma_start(boc_t[:], boc[:])
        cop = psum([BL, H])
        for k in range(HC):
            nc.tensor.matmul(cop[:], hT[k][:], WocT_c[k][:],
                             start=(k == 0), stop=False)
        for k in range(HC):
            nc.tensor.matmul(cop[:], ctxT[k][:], WocT_c[HC + k][:],
                             start=False, stop=False)
        nc.tensor.matmul(cop[:], ones_t[:, :BL], boc_t[:],
                         start=False, stop=True)
        co_t = sbs.tile([BL, H], F32, bufs=1)
        nc.scalar.activation(co_t[:], cop[:], AF.Tanh)
        nc.sync.dma_start(co_out[:], co_t[:])

        # AllGather co -> co_all [64, 512]
        co_bb = dr.tile([BL, H], F32)
        coall_bb = dr.tile([B, H], F32)
        nc.sync.dma_start(co_bb[:], co_t[:])
        nc.gpsimd.collective_compute(
            "AllGather", ALU.bypass,
            replica_groups=[list(range(N_CORES))],
            ins=[co_bb[:].opt()], outs=[coall_bb[:].opt()])
        co_all = sbs.tile([B, H], F32, bufs=1)
        nc.sync.dma_start(co_all[:], coall_bb[:])
        coallT = []
        for k in range(HC):
            tp = psum([128, B])
            nc.tensor.transpose(tp[:], co_all[:, k * 128:(k + 1) * 128],
                                identt[:B, :B])
            ct = sbs.tile([128, B], F32R, tag="coallT", bufs=HC,
                          name=f"coallT{k}")
            nc.scalar.copy(ct[:], tp[:])
            coallT.append(ct)

        # logits [64, VS] in 500-wide chunks; exp-sum partials
        bout_t = sbs.tile([1, VS], F32R, bufs=1)
        nc.sync.dma_start(bout_t[:], bout[:])
        logits_sb = sb.tile([B, VS], F32)
        zparts = []
        for n in range(NV):
            lp = psum([B, 500])
            for k in range(HC):
                nc.tensor.matmul(lp[:], coallT[k][:], wout_tiles[(k, n)][:],
                                 start=(k == 0), stop=False)
            nc.tensor.matmul(lp[:], ones_t[:, :B],
                             bout_t[:, n * 500:(n + 1) * 500],
                             start=False, stop=True)
            nsl = slice(n * 500, (n + 1) * 500)
            nc.vector.tensor_copy(logits_sb[:, nsl], lp[:])
            escr = sbs.tile([B, 500], F32, tag="escr", name=f"escr{n}")
            zp = sbs.tile([B, 1], F32, tag=f"zp{n}", bufs=1, name=f"zp{n}")
            nc.scalar.activation(escr[:], lp[:], AF.Exp, accum_out=zp[:])
            zparts.append(zp)
        zsum = sbs.tile([B, 1], F32, bufs=1)
        nc.vector.tensor_add(zsum[:], zparts[0][:], zparts[1][:])
        for n in range(2, NV):
            nc.vector.tensor_add(zsum[:], zsum[:], zparts[n][:])

        # AllReduce softmax denominator
        z_bb = dr.tile([B, 1], F32)
        zr_bb = dr.tile([B, 1], F32)
        nc.sync.dma_start(z_bb[:], zsum[:])
        nc.gpsimd.collective_compute(
            "AllReduce", ALU.add,
            replica_groups=[list(range(N_CORES))],
            ins=[z_bb[:].opt()], outs=[zr_bb[:].opt()])
        z_all = sbs.tile([B, 1], F32, bufs=1)
        nc.sync.dma_start(z_all[:], zr_bb[:])
        logz = sbs.tile([B, 1], F32, bufs=1)
        nc.scalar.activation(logz[:], z_all[:], AF.Ln)
        nc.vector.tensor_scalar(out=logits_sb[:], in0=logits_sb[:],
                                scalar1=logz[:], scalar2=None,
                                op0=ALU.subtract)
        nc.sync.dma_start(out_slice[:], logits_sb[:])
    nc.compile()
    return nc


_NC_CACHE = {}


def _get_nc(nL=L):
    if nL not in _NC_CACHE:
        _NC_CACHE[nL] = _build(nL)
    return _NC_CACHE[nL]


def _make_in_maps(inputs, nL=L):
    f32 = lambda a: np.ascontiguousarray(np.asarray(a), dtype=np.float32)
    enc = f32(inputs["encoder_outputs"])[:nL]
    loc = f32(inputs["last_output_context"])[0]
    hprev = f32(inputs["last_hidden"])[0]
    cprev = f32(inputs["last_cell_state"])[0]
    idx = np.ascontiguousarray(np.asarray(inputs["input"]).reshape(B, 1),
                               dtype=np.int32)
    W_ih, W_hh = f32(inputs["W_ih"]), f32(inputs["W_hh"])
    b_ih, b_hh = f32(inputs["b_ih"]), f32(inputs["b_hh"])
    W_attn, b_attn = f32(inputs["W_attn"]), f32(inputs["b_attn"])
    W_oc, b_oc = f32(inputs["W_oc"]), f32(inputs["b_oc"])
    W_out, b_out = f32(inputs["W_out"]), f32(inputs["b_out"])

    GW = min(512, nL * BL)
    shared = {
        "emb": f32(inputs["emb"]),
        "WihT": np.ascontiguousarray(W_ih.T),
        "bihh": np.ascontiguousarray((b_ih + b_hh)[None]),
        "WhhT": np.ascontiguousarray(W_hh.T),
        "WhT": np.ascontiguousarray(W_attn.T[:H]),
        "battn": np.ascontiguousarray(b_attn[None]),
        "WencT": np.ascontiguousarray(W_attn.T[H:]),
        "beta": f32(inputs["beta"]),
        "WocT": np.ascontiguousarray(W_oc.T),
        "boc": np.ascontiguousarray(b_oc[None]),
        "maskd": np.ascontiguousarray(
            (np.arange(GW)[None, :] % BL == np.arange(BL)[:, None])
            .astype(np.float32)),
        "ones128": np.ones((1, 128), dtype=np.float32),
        "ident": np.eye(128, dtype=np.float32),
    }
    WoutT = np.ascontiguousarray(W_out.T)
    in_maps = []
    for c in range(N_CORES):
        bsl = slice(c * BL, (c + 1) * BL)
        vsl = slice(c * VS, (c + 1) * VS)
        m = dict(shared)
        m["encT"] = np.ascontiguousarray(
            enc[:, bsl, :].transpose(2, 0, 1).reshape(H, nL * BL))
        m["tok_idx"] = idx[bsl]
        m["locT"] = np.ascontiguousarray(loc[bsl].T)
        m["hprevT"] = np.ascontiguousarray(hprev[bsl].T)
        m["cprev"] = np.ascontiguousarray(cprev[bsl])
        m["WoutT"] = np.ascontiguousarray(WoutT[:, vsl])
        m["bout"] = np.ascontiguousarray(b_out[None, vsl])
        in_maps.append(m)
    return in_maps


def _run(inputs, nL=L, trace=False):
    nc = _get_nc(nL)
    in_maps = _make_in_maps(inputs, nL)
    res = run_bass_kernel_spmd(nc, in_maps, core_ids=list(range(N_CORES)),
                               trace=trace)
    out = np.concatenate([res.results[c]["out_slice"]
                          for c in range(N_CORES)], axis=1)
    co = np.concatenate([res.results[c]["co_out"]
                         for c in range(N_CORES)], axis=0)[None]
    h = np.concatenate([res.results[c]["h_out"]
                        for c in range(N_CORES)], axis=0)[None]
    cst = np.concatenate([res.results[c]["c_out"]
                          for c in range(N_CORES)], axis=0)[None]
    return (out, co, h, cst), res


def kernel(**inputs):
    outs, _ = _run(inputs, nL=L, trace=False)
    return outs


# revision 11
# speedup vs baseline: 1.0659x; 1.0659x over previous
"""AttnDecoderRNN single-step decoder on 8 TRN2 NeuronCores.

Sharding: data-parallel over batch B=64 -> 8 rows/core for the
LSTM + additive attention (the dominant compute, the L*B*H*H attention
projection, is evenly split); vocab-tensor-parallel for the output
projection (W_out split into 8 slices of 4000 rows). Cross-core
traffic: AllGather of co (16KB/core) + AllReduce of the softmax
denominator partials (256B).

Device layouts (host pre-stages):
  encT  [H=512, L*8=8192]  encoder slice transposed, lb = l*8 + b
  all matmul operands typed float32r (4x PE throughput, ~1.5e-4 rel err)
"""
import sys

if "/opt/trn_rl_repo" not in sys.path:
    sys.path.insert(0, "/opt/trn_rl_repo")

import contextlib

import numpy as np

import concourse.bass as bass
import concourse.tile as tile
from concourse import bacc, mybir
from concourse.bass_utils import run_bass_kernel_spmd

F32 = mybir.dt.float32
F32R = mybir.dt.float32r
I32 = mybir.dt.int32
AF = mybir.ActivationFunctionType
ALU = mybir.AluOpType

N_CORES = 8
B, H, E, V, L = 64, 512, 256, 32000, 1024
BL = B // N_CORES          # 8 batch rows per core
VS = V // N_CORES          # 4000 vocab rows per core
HC = H // 128              # 4 h chunks
EC = E // 128              # 2 embedding chunks


def _build(nL=L):
    """Build the SPMD kernel. nL parameterizes seq len (small for sim)."""
    LBL = nL * BL            # local (l, b) row count
    NG = max(1, LBL // 512)  # lb groups
    GW = LBL // NG           # group width (512 normally)
    NV = VS // 500           # 8 vocab N-chunks of 500

    nc = bacc.Bacc("TRN2", target_bir_lowering=False, debug=False,
                   num_devices=N_CORES)
    dram = lambda n, s, d: nc.dram_tensor(n, s, d, kind="ExternalInput").ap()
    outd = lambda n, s: nc.dram_tensor(n, s, F32, kind="ExternalOutput").ap()

    # inputs
    encT = dram("encT", [H, LBL], F32R)
    tok_idx = dram("tok_idx", [BL, 1], I32)
    emb = dram("emb", [V, E], F32)
    locT = dram("locT", [H, BL], F32R)
    hprevT = dram("hprevT", [H, BL], F32R)
    cprev = dram("cprev", [BL, H], F32)
    WihT = dram("WihT", [E + H, 4 * H], F32R)
    bihh = dram("bihh", [1, 4 * H], F32R)
    WhhT = dram("WhhT", [H, 4 * H], F32R)
    WhT = dram("WhT", [H, H], F32R)
    battn = dram("battn", [1, H], F32R)
    WencT = dram("WencT", [H, H], F32R)
    beta = dram("beta", [H, 1], F32R)
    WocT = dram("WocT", [2 * H, H], F32R)
    boc = dram("boc", [1, H], F32R)
    WoutT = dram("WoutT", [H, VS], F32R)
    bout = dram("bout", [1, VS], F32R)
    maskd = dram("maskd", [BL, GW], F32R)      # mask[b, f] = (f % 8 == b)
    ones128 = dram("ones128", [1, 128], F32R)
    ident = dram("ident", [128, 128], F32)

    # outputs
    out_slice = outd("out_slice", [B, VS])
    co_out = outd("co_out", [BL, H])
    h_out = outd("h_out", [BL, H])
    c_out = outd("c_out", [BL, H])

    with tile.TileContext(nc) as tc, contextlib.ExitStack() as ctx:
        sb = ctx.enter_context(tc.tile_pool(name="sb", bufs=1))
        sbs = ctx.enter_context(tc.tile_pool(name="sbs", bufs=2))
        enc_pool = ctx.enter_context(tc.tile_pool(name="encp", bufs=2))
        tanh_pool = ctx.enter_context(tc.tile_pool(name="tanhp", bufs=2))
        w_pool = ctx.enter_context(tc.tile_pool(name="wp", bufs=4))
        wout_pool = ctx.enter_context(tc.tile_pool(name="woutp", bufs=1))
        ps = ctx.enter_context(tc.tile_pool(name="ps", bufs=1, space="PSUM"))
        dr = ctx.enter_context(tc.tile_pool(name="dr", bufs=1, space="DRAM"))

        _pid = [0]

        def psum(shape, tag="mmO", bufs=5):
            _pid[0] += 1
            return ps.tile(shape, F32, space="PSUM", tag=tag, bufs=bufs,
                           name=f"ps_{tag}_{_pid[0]}")

        # ---- constants / small resident tiles ----
        identt = sb.tile([128, 128], F32)
        nc.gpsimd.dma_start(identt[:], ident[:])
        ones_t = sb.tile([1, 128], F32R)
        nc.gpsimd.dma_start(ones_t[:], ones128[:])
        mask_t = sb.tile([BL, GW], F32R)
        nc.gpsimd.dma_start(mask_t[:], maskd[:])
        beta_c = []
        for k in range(HC):
            bt = sb.tile([128, 1], F32R, tag=f"beta{k}", name=f"beta{k}")
            nc.gpsimd.dma_start(bt[:], beta[k * 128:(k + 1) * 128, :])
            beta_c.append(bt)
        WencT_c = []
        for k in range(HC):
            row = []
            for m in range(HC):
                wt = sb.tile([128, 128], F32R, tag=f"wenc{k}{m}",
                             name=f"wenc{k}{m}")
                nc.gpsimd.dma_start(
                    wt[:], WencT[k * 128:(k + 1) * 128,
                                 m * 128:(m + 1) * 128])
                row.append(wt)
            WencT_c.append(row)

        # collective firmware warmup (off critical path)
        wu_a = dr.tile([8, 4], F32)
        wu_b = dr.tile([8, 4], F32)
        wu_s = sbs.tile([8, 4], F32, bufs=1)
        nc.gpsimd.memset(wu_s[:], 0.0)
        nc.gpsimd.dma_start(wu_a[:], wu_s[:])
        nc.gpsimd.collective_compute(
            "AllReduce", ALU.add, replica_groups=[list(range(N_CORES))],
            ins=[wu_a[:].opt()], outs=[wu_b[:].opt()])

        # ---- phase 0: embedding gather + LSTM + h-term ----
        idx_t = sbs.tile([BL, 1], I32, bufs=1)
        nc.sync.dma_start(idx_t[:], tok_idx[:])
        tok_t = sbs.tile([BL, E], F32, bufs=1)
        nc.gpsimd.indirect_dma_start(
            out=tok_t[:], out_offset=None, in_=emb[:],
            in_offset=bass.IndirectOffsetOnAxis(ap=idx_t[:, :1], axis=0))

        # xT chunks (f32r): tok (2 via PE transpose), loc (4 from DRAM)
        xT = []
        for k in range(EC):
            tp = psum([128, BL])
            nc.tensor.transpose(tp[:], tok_t[:, k * 128:(k + 1) * 128],
                                identt[:BL, :BL])
            xt = sbs.tile([128, BL], F32R, tag="xT", bufs=EC + HC,
                          name=f"xT{k}")
            nc.scalar.copy(xt[:], tp[:])
            xT.append(xt)
        for k in range(HC):
            xt = sbs.tile([128, BL], F32R, tag="xT", bufs=EC + HC,
                          name=f"xT{EC + k}")
            nc.gpsimd.dma_start(xt[:], locT[k * 128:(k + 1) * 128, :])
            xT.append(xt)
        hpv = []
        for k in range(HC):
            ht = sbs.tile([128, BL], F32R, tag="hprevT", bufs=HC,
                          name=f"hprevT{k}")
            nc.gpsimd.dma_start(ht[:], hprevT[k * 128:(k + 1) * 128, :])
            hpv.append(ht)

        # gates[b, 4H] in 4 psum tiles [8, 512] (i, f, g, o)
        gact = []
        for n in range(4):
            gp = psum([BL, H])
            nsl = slice(n * H, (n + 1) * H)
            for k in range(EC + HC):   # W_ih part
                wt = w_pool.tile([128, H], F32R, tag="wih", name=f"wih{n}_{k}")
                nc.gpsimd.dma_start(wt[:], WihT[k * 128:(k + 1) * 128, nsl])
                nc.tensor.matmul(gp[:], xT[k][:], wt[:],
                                 start=(k == 0), stop=False)
            for k in range(HC):        # W_hh part
                wt = w_pool.tile([128, H], F32R, tag="whh", name=f"whh{n}_{k}")
                nc.gpsimd.dma_start(wt[:], WhhT[k * 128:(k + 1) * 128, nsl])
                nc.tensor.matmul(gp[:], hpv[k][:], wt[:],
                                 start=False, stop=False)
            bt = w_pool.tile([1, H], F32R, tag="bias", name=f"bias{n}")
            nc.gpsimd.dma_start(bt[:], bihh[:, nsl])
            nc.tensor.matmul(gp[:], ones_t[:, :BL], bt[:],
                             start=False, stop=True)
            ga = sbs.tile([BL, H], F32, tag="gate", bufs=4, name=f"gate{n}")
            nc.scalar.activation(ga[:], gp[:],
                                 AF.Tanh if n == 2 else AF.Sigmoid)
            gact.append(ga)

        cprev_t = sbs.tile([BL, H], F32, bufs=1)
        nc.gpsimd.dma_start(cprev_t[:], cprev[:])
        t1 = sbs.tile([BL, H], F32, bufs=1)
        nc.vector.tensor_mul(t1[:], gact[1][:], cprev_t[:])   # f * c_prev
        t2 = sbs.tile([BL, H], F32, bufs=1)
        nc.vector.tensor_mul(t2[:], gact[0][:], gact[2][:])   # i * g
        c_new = sbs.tile([BL, H], F32, bufs=1)
        nc.vector.tensor_add(c_new[:], t1[:], t2[:])
        nc.sync.dma_start(c_out[:], c_new[:])
        ctanh = sbs.tile([BL, H], F32, bufs=1)
        nc.scalar.activation(ctanh[:], c_new[:], AF.Tanh)
        h_new = sbs.tile([BL, H], F32, bufs=1)
        nc.vector.tensor_mul(h_new[:], gact[3][:], ctanh[:])  # o * tanh(c)
        nc.sync.dma_start(h_out[:], h_new[:])

        # hT chunks (f32r), used by hp and co matmuls
        hT = []
        for k in range(HC):
            tp = psum([128, BL])
            nc.tensor.transpose(tp[:], h_new[:, k * 128:(k + 1) * 128],
                                identt[:BL, :BL])
            ht = sbs.tile([128, BL], F32R, tag="hT", bufs=HC, name=f"hT{k}")
            nc.scalar.copy(ht[:], tp[:])
            hT.append(ht)

        # hp[b, h_out] = h @ W_h.T + b_attn
        WhT_c = []
        for k in range(HC):
            wt = sbs.tile([128, H], F32R, tag="whT", bufs=2, name=f"whT{k}")
            nc.gpsimd.dma_start(wt[:], WhT[k * 128:(k + 1) * 128, :])
            WhT_c.append(wt)
        battn_t = sbs.tile([1, H], F32R, bufs=1)
        nc.gpsimd.dma_start(battn_t[:], battn[:])
        hpp = psum([BL, H])
        for k in range(HC):
            nc.tensor.matmul(hpp[:], hT[k][:], WhT_c[k][:],
                             start=(k == 0), stop=False)
        nc.tensor.matmul(hpp[:], ones_t[:, :BL], battn_t[:],
                         start=False, stop=True)
        hp_t = sbs.tile([BL, H], F32R, bufs=1)
        nc.scalar.copy(hp_t[:], hpp[:])

        # context accumulators + softmax running sum
        ctx_buf = []
        for m in range(HC):
            cb = sb.tile([128, BL * NG], F32, tag=f"ctxb{m}", name=f"ctxb{m}")
            ctx_buf.append(cb)
        sum_buf = sb.tile([1, BL * NG], F32)

        # WoutT tiles: prefetched during the attention loop
        wout_tiles = {}

        NRES = NV // 2   # resident n-chunks; rest streamed in logits loop

        def prefetch_wout(j):
            if j >= HC * NRES:
                return
            k, n = divmod(j, NRES)
            wt = wout_pool.tile([128, 500], F32R, tag=f"wo{j}", name=f"wo{j}")
            nc.gpsimd.dma_start(
                wt[:], WoutT[k * 128:(k + 1) * 128, n * 500:(n + 1) * 500])
            wout_tiles[(k, n)] = wt

        # ---- phase 1: attention main loop over lb groups ----
        for g in range(NG):
            gsl = slice(g * GW, (g + 1) * GW)
            enc_g = []
            for k in range(HC):
                et = enc_pool.tile([128, GW], F32R, tag=f"enc{k}",
                                   name=f"enc{k}_{g}")
                nc.sync.dma_start(et[:], encT[k * 128:(k + 1) * 128, gsl])
                enc_g.append(et)
            tanh_g = []
            for m in range(HC):
                ep = psum([128, GW])
                for k in range(HC):
                    nc.tensor.matmul(ep[:], WencT_c[k][m][:], enc_g[k][:],
                                     start=(k == 0), stop=False)
                nc.tensor.matmul(ep[:], hp_t[:, m * 128:(m + 1) * 128],
                                 mask_t[:], start=False, stop=True)
                tt = tanh_pool.tile([128, GW], F32R, tag=f"tanh{m}",
                                    name=f"tanh{m}_{g}")
                nc.scalar.activation(tt[:], ep[:], AF.Tanh)
                tanh_g.append(tt)
            bp = psum([1, GW], tag="betaO", bufs=3)
            for m in range(HC):
                nc.tensor.matmul(bp[:], beta_c[m][:], tanh_g[m][:],
                                 start=(m == 0), stop=(m == HC - 1))
            exp_row = sbs.tile([1, GW], F32R, tag="exprow", name=f"exprow{g}")
            nc.scalar.activation(exp_row[:], bp[:], AF.Exp)
            # running softmax denominator (per b)
            nc.vector.tensor_reduce(
                sum_buf[:, g * BL:(g + 1) * BL],
                exp_row[:].bitcast(F32).rearrange("p (l b) -> p b l", b=BL),
                axis=mybir.AxisListType.X, op=ALU.add)
            # replicate exp across partitions (PE K=1 ones-matmul)
            arep = psum([128, GW], tag="betaO", bufs=3)
            nc.tensor.matmul(arep[:], ones_t[:], exp_row[:],
                             start=True, stop=True)
            for m in range(HC):
                prod = sbs.tile([128, GW], F32, tag="prod",
                                name=f"prod{m}_{g}")
                nc.vector.tensor_mul(prod[:], enc_g[m][:].bitcast(F32),
                                     arep[:])
                nc.vector.tensor_reduce(
                    ctx_buf[m][:, g * BL:(g + 1) * BL],
                    prod[:].rearrange("p (l b) -> p b l", b=BL),
                    axis=mybir.AxisListType.X, op=ALU.add)
            prefetch_wout(2 * g)
            prefetch_wout(2 * g + 1)

        for j in range(2 * NG, HC * NRES):
            prefetch_wout(j)

        # ---- phase 2: normalize context, co, exchange, logits ----
        sum_acc = sbs.tile([1, BL], F32, bufs=1)
        nc.vector.tensor_reduce(
            sum_acc[:], sum_buf[:].rearrange("p (g b) -> p b g", b=BL),
            axis=mybir.AxisListType.X, op=ALU.add)
        ctx_acc = []
        for m in range(HC):
            ca = sbs.tile([128, BL], F32, tag="ctxacc", bufs=HC,
                          name=f"ctxacc{m}")
            nc.vector.tensor_reduce(
                ca[:], ctx_buf[m][:].rearrange("p (g b) -> p b g", b=BL),
                axis=mybir.AxisListType.X, op=ALU.add)
            ctx_acc.append(ca)
        inv1 = sbs.tile([1, BL], F32, bufs=1)
        nc.vector.reciprocal(inv1[:], sum_acc[:])
        inv1r = sbs.tile([1, BL], F32R, bufs=1)
        nc.scalar.copy(inv1r[:], inv1[:])
        irp = psum([128, BL])
        nc.tensor.matmul(irp[:], ones_t[:], inv1r[:], start=True, stop=True)
        inv_rep = sbs.tile([128, BL], F32, bufs=1)
        nc.scalar.copy(inv_rep[:], irp[:])
        ctxT = []
        for m in range(HC):
            cn = sbs.tile([128, BL], F32, tag="ctxn", name=f"ctxn{m}")
            nc.vector.tensor_mul(cn[:], ctx_acc[m][:], inv_rep[:])
            cnr = sbs.tile([128, BL], F32R, tag="ctxnr", bufs=HC,
                           name=f"ctxnr{m}")
            nc.scalar.copy(cnr[:], cn[:])
            ctxT.append(cnr)

        # co = tanh([h; ctx] @ W_oc.T + b_oc)
        WocT_c = []
        for k in range(2 * HC):
            wt = sbs.tile([128, H], F32R, tag="wocT", bufs=3,
                          name=f"wocT{k}")
            nc.gpsimd.dma_start(wt[:], WocT[k * 128:(k + 1) * 128, :])
            WocT_c.append(wt)
        boc_t = sbs.tile([1, H], F32R, bufs=1)
        nc.gpsimd.dma_start(boc_t[:], boc[:])
        cop = psum([BL, H])
        for k in range(HC):
            nc.tensor.matmul(cop[:], hT[k][:], WocT_c[k][:],
                             start=(k == 0), stop=False)
        for k in range(HC):
            nc.tensor.matmul(cop[:], ctxT[k][:], WocT_c[HC + k][:],
                             start=False, stop=False)
        nc.tensor.matmul(cop[:], ones_t[:, :BL], boc_t[:],
                         start=False, stop=True)
        co_t = sbs.tile([BL, H], F32, bufs=1)
        nc.scalar.activation(co_t[:], cop[:], AF.Tanh)
        nc.sync.dma_start(co_out[:], co_t[:])

        # AllGather co -> co_all [64, 512]
        co_bb = dr.tile([BL, H], F32)
        coall_bb = dr.tile([B, H], F32)
        nc.sync.dma_start(co_bb[:], co_t[:])
        nc.gpsimd.collective_compute(
            "AllGather", ALU.bypass,
            replica_groups=[list(range(N_CORES))],
            ins=[co_bb[:].opt()], outs=[coall_bb[:].opt()])
        co_all = sbs.tile([B, H], F32, bufs=1)
        nc.sync.dma_start(co_all[:], coall_bb[:])
        coallT = []
        for k in range(HC):
            tp = psum([128, B])
            nc.tensor.transpose(tp[:], co_all[:, k * 128:(k + 1) * 128],
                                identt[:B, :B])
            ct = sbs.tile([128, B], F32R, tag="coallT", bufs=HC,
                          name=f"coallT{k}")
            nc.scalar.copy(ct[:], tp[:])
            coallT.append(ct)

        # logits [64, VS] in 500-wide chunks; exp-sum partials
        bout_t = sbs.tile([1, VS], F32R, bufs=1)
        nc.gpsimd.dma_start(bout_t[:], bout[:])
        logits_sb = sb.tile([B, VS], F32)
        zparts = []
        for n in range(NV):
            if n >= NRES:
                for k in range(HC):
                    wt = wout_pool.tile([128, 500], F32R, tag=f"wos{k}",
                                        bufs=2, name=f"wos{k}_{n}")
                    nc.gpsimd.dma_start(
                        wt[:], WoutT[k * 128:(k + 1) * 128,
                                     n * 500:(n + 1) * 500])
                    wout_tiles[(k, n)] = wt
            lp = psum([B, 500])
            for k in range(HC):
                nc.tensor.matmul(lp[:], coallT[k][:], wout_tiles[(k, n)][:],
                                 start=(k == 0), stop=False)
            nc.tensor.matmul(lp[:], ones_t[:, :B],
                             bout_t[:, n * 500:(n + 1) * 500],
                             start=False, stop=True)
            nsl = slice(n * 500, (n + 1) * 500)
            nc.vector.tensor_copy(logits_sb[:, nsl], lp[:])
            escr = sbs.tile([B, 500], F32, tag="escr", bufs=1,
                            name=f"escr{n}")
            zp = sbs.tile([B, 1], F32, tag=f"zp{n}", bufs=1, name=f"zp{n}")
            nc.scalar.activation(escr[:], lp[:], AF.Exp, accum_out=zp[:])
            zparts.append(zp)
        zsum = sbs.tile([B, 1], F32, bufs=1)
        nc.vector.tensor_add(zsum[:], zparts[0][:], zparts[1][:])
        for n in range(2, NV):
            nc.vector.tensor_add(zsum[:], zsum[:], zparts[n][:])

        # AllReduce softmax denominator
        z_bb = dr.tile([B, 1], F32)
        zr_bb = dr.tile([B, 1], F32)
        nc.sync.dma_start(z_bb[:], zsum[:])
        nc.gpsimd.collective_compute(
            "AllReduce", ALU.add,
            replica_groups=[list(range(N_CORES))],
            ins=[z_bb[:].opt()], outs=[zr_bb[:].opt()])
        z_all = sbs.tile([B, 1], F32, bufs=1)
        nc.sync.dma_start(z_all[:], zr_bb[:])
        logz = sbs.tile([B, 1], F32, bufs=1)
        nc.scalar.activation(logz[:], z_all[:], AF.Ln)
        nc.vector.tensor_scalar(out=logits_sb[:], in0=logits_sb[:],
                                scalar1=logz[:], scalar2=None,
                                op0=ALU.subtract)
        nc.sync.dma_start(out_slice[:], logits_sb[:])
    nc.compile()
    return nc


_NC_CACHE = {}


def _get_nc(nL=L):
    if nL not in _NC_CACHE:
        _NC_CACHE[nL] = _build(nL)
    return _NC_CACHE[nL]


def _make_in_maps(inputs, nL=L):
    f32 = lambda a: np.ascontiguousarray(np.asarray(a), dtype=np.float32)
    enc = f32(inputs["encoder_outputs"])[:nL]
    loc = f32(inputs["last_output_context"])[0]
    hprev = f32(inputs["last_hidden"])[0]
    cprev = f32(inputs["last_cell_state"])[0]
    idx = np.ascontiguousarray(np.asarray(inputs["input"]).reshape(B, 1),
                               dtype=np.int32)
    W_ih, W_hh = f32(inputs["W_ih"]), f32(inputs["W_hh"])
    b_ih, b_hh = f32(inputs["b_ih"]), f32(inputs["b_hh"])
    W_attn, b_attn = f32(inputs["W_attn"]), f32(inputs["b_attn"])
    W_oc, b_oc = f32(inputs["W_oc"]), f32(inputs["b_oc"])
    W_out, b_out = f32(inputs["W_out"]), f32(inputs["b_out"])

    GW = min(512, nL * BL)
    shared = {
        "emb": f32(inputs["emb"]),
        "WihT": np.ascontiguousarray(W_ih.T),
        "bihh": np.ascontiguousarray((b_ih + b_hh)[None]),
        "WhhT": np.ascontiguousarray(W_hh.T),
        "WhT": np.ascontiguousarray(W_attn.T[:H]),
        "battn": np.ascontiguousarray(b_attn[None]),
        "WencT": np.ascontiguousarray(W_attn.T[H:]),
        "beta": f32(inputs["beta"]),
        "WocT": np.ascontiguousarray(W_oc.T),
        "boc": np.ascontiguousarray(b_oc[None]),
        "maskd": np.ascontiguousarray(
            (np.arange(GW)[None, :] % BL == np.arange(BL)[:, None])
            .astype(np.float32)),
        "ones128": np.ones((1, 128), dtype=np.float32),
        "ident": np.eye(128, dtype=np.float32),
    }
    WoutT = np.ascontiguousarray(W_out.T)
    in_maps = []
    for c in range(N_CORES):
        bsl = slice(c * BL, (c + 1) * BL)
        vsl = slice(c * VS, (c + 1) * VS)
        m = dict(shared)
        m["encT"] = np.ascontiguousarray(
            enc[:, bsl, :].transpose(2, 0, 1).reshape(H, nL * BL))
        m["tok_idx"] = idx[bsl]
        m["locT"] = np.ascontiguousarray(loc[bsl].T)
        m["hprevT"] = np.ascontiguousarray(hprev[bsl].T)
        m["cprev"] = np.ascontiguousarray(cprev[bsl])
        m["WoutT"] = np.ascontiguousarray(WoutT[:, vsl])
        m["bout"] = np.ascontiguousarray(b_out[None, vsl])
        in_maps.append(m)
    return in_maps


def _run(inputs, nL=L, trace=False):
    nc = _get_nc(nL)
    in_maps = _make_in_maps(inputs, nL)
    res = run_bass_kernel_spmd(nc, in_maps, core_ids=list(range(N_CORES)),
                               trace=trace)
    out = np.concatenate([res.results[c]["out_slice"]
                          for c in range(N_CORES)], axis=1)
    co = np.concatenate([res.results[c]["co_out"]
                         for c in range(N_CORES)], axis=0)[None]
    h = np.concatenate([res.results[c]["h_out"]
                        for c in range(N_CORES)], axis=0)[None]
    cst = np.concatenate([res.results[c]["c_out"]
                          for c in range(N_CORES)], axis=0)[None]
    return (out, co, h, cst), res


def kernel(**inputs):
    outs, _ = _run(inputs, nL=L, trace=False)
    return outs
